# revision 2
# baseline (speedup 1.0000x reference)
"""Bahdanau additive attention + LayerNorm, distributed over 8 TRN2 NeuronCores.

Data parallel over batch: each core handles 128 of the 1024 batch rows.
Per core (B=128, S=1024, D=H=128), v2 dataflow:
  - Host supplies TWO copies of h_s: natural-layout bf16 (context operand) and
    a transposed fp8-e4m3 copy h_sT (projection operand). This removes the
    on-device TensorE transpose pass and its DVE psum evacuations entirely,
    and cuts HBM traffic from 64 MiB (f32) to 48 MiB.
  - Projection runs as U_bf16^T @ hsT_fp8 into PSUM; the per-row ht_proj bias
    is added during the PSUM->SBUF evacuation on DVE (tensor_scalar), so the
    ACT engine can run tanh batched 4 rows per instruction (amortizing the
    ~300-cycle per-activation fixed cost) instead of one biased call per row.
  - Scores: tanh chunks as matmul weights vs V (1-col rhs), landing across
    partitions; softmax unnormalized (deferred divide in the epilogue).
  - Context: nat chunks as weights vs exp column, accumulated in PSUM.
"""

import numpy as np
from contextlib import ExitStack

import concourse.bass as bass
import concourse.mybir as mybir
from concourse.bass_utils import run_bass_kernel_spmd
from concourse.tile import TileContext
from concourse.vector_clock import ScopedClock, VectorClock
from concourse.masks import make_identity

# ---------------------------------------------------------------------------
# Workaround for walrus "Too many sync wait commands" on the TileContext final
# Drain: put the end-of-kernel semaphore waits on individual nops (engine
# instructions execute in order, so a bare drain afterwards is equivalent).
# ---------------------------------------------------------------------------


def _patched_drain_and_barrier(self, tick_clock, wait_clock):
    gc = tick_clock.global_clock
    for i, t in enumerate(list(gc)):
        if t > 0:
            pc = VectorClock()
            for _ in range(t):
                pc.advance(i)
            nop_i = self.nc.sync.nop(hint=f"drainwait{i}", nofuse=True)
            wait_clock.add_sem_waits(nop_i.ins, ScopedClock({None: pc}))
    self.nc.sync.drain()
    self.nc.all_engine_barrier()
    assert self.sems is not None
    popped = self.nc._tile_sem_poison_stack.pop()
    assert popped is self._sem_poison
    self.nc.clear_and_free_semaphores(list(self.sems.allocated().values()))
    self.nc.all_engine_barrier()


TileContext._drain_and_barrier = _patched_drain_and_barrier

# ---------------------------------------------------------------------------

NCORES = 8
B = 128          # batch rows per core (1024 / 8)
S = 1024
D = 128
H = 128
EPS = 1e-3

F32 = mybir.dt.float32
BF16 = mybir.dt.bfloat16
F8 = mybir.dt.float8e4

NCH = S // 128   # 8 s-chunks of 128
BLK = 4          # rows per block (one tanh batch / one DMA tile)
NB = B // BLK    # 32 blocks
GRP = 8          # rows per softmax group (= 2 blocks)
NG = B // GRP    # 16 groups
LAG = 2          # blocks between proj emission and scores emission

Tanh = mybir.ActivationFunctionType.Tanh
Exp = mybir.ActivationFunctionType.Exp
Ln = mybir.ActivationFunctionType.Ln
ADD = mybir.AluOpType.add
SUB = mybir.AluOpType.subtract
MULT = mybir.AluOpType.mult
AX_X = mybir.AxisListType.X


def _build(debug=False):
    nc = bass.Bass()
    h_t_e = nc.declare_dram_parameter("h_t", [B, D], F32, isOutput=False)
    # both h_s copies come host-prearranged in the exact SBUF tile layout:
    #   h_s  [p, blk, t, r, d] bf16  (s = 8p + r, row b = 4*blk + t)
    #   h_sT [d, blk, t, j]    fp8   (col j = c*128 + p <-> s = 8p + c)
    # so each per-block DMA is one contiguous chunk per partition.
    h_s_e = nc.declare_dram_parameter("h_s", [128, NB, BLK, NCH, D], BF16,
                                      isOutput=False)
    h_sT_e = nc.declare_dram_parameter("h_sT", [128, NB, BLK, S], F8,
                                       isOutput=False)
    W_a_e = nc.declare_dram_parameter("W_a", [D, H], F32, isOutput=False)
    U_a_e = nc.declare_dram_parameter("U_a", [D, H], BF16, isOutput=False)
    V_a_e = nc.declare_dram_parameter("V_a", [H, 1], BF16, isOutput=False)
    W_c_e = nc.declare_dram_parameter("W_c", [2 * D, H], F32, isOutput=False)
    b_c_e = nc.declare_dram_parameter("b_c", [H], F32, isOutput=False)
    gamma_e = nc.declare_dram_parameter("gamma", [H], F32, isOutput=False)
    beta_e = nc.declare_dram_parameter("beta", [H], F32, isOutput=False)
    out_e = nc.declare_dram_parameter("out", [B, H], F32, isOutput=True)
    if debug:
        dbg_energy = nc.declare_dram_parameter("dbg_energy", [H, S], F32, isOutput=True)
        dbg_tanh = nc.declare_dram_parameter("dbg_tanh", [H, S], F32, isOutput=True)
        dbg_exp = nc.declare_dram_parameter("dbg_exp", [128, 64], F32, isOutput=True)
        dbg_sums = nc.declare_dram_parameter("dbg_sums", [1, B], F32, isOutput=True)
        dbg_ctx = nc.declare_dram_parameter("dbg_ctx", [D, B], F32, isOutput=True)

    with TileContext(nc) as tc, ExitStack() as ctx:
        consts = ctx.enter_context(tc.tile_pool(name="consts", bufs=1))
        # nat lifetime is ~9-10 slots (DMA at k-3, freed by ctx at k+6); at
        # bufs=10 the pool sits exactly at its steady-state depth, so every
        # nat DMA start gates on a ctx-side buffer release. Two spare
        # buffers decouple the DMA stream from compute jitter.
        nat_pool = ctx.enter_context(tc.tile_pool(name="nat", bufs=12))
        hsT_pool = ctx.enter_context(tc.tile_pool(name="hsT", bufs=6))
        en_pool = ctx.enter_context(tc.tile_pool(name="energy", bufs=3))
        tanh_pool = ctx.enter_context(tc.tile_pool(name="tanh", bufs=4))
        exp_pool = ctx.enter_context(tc.tile_pool(name="expT", bufs=3))
        small = ctx.enter_context(tc.tile_pool(name="small", bufs=4))
        pe_psum = ctx.enter_context(tc.tile_pool(name="pe_psum", bufs=2, space="PSUM"))
        sc_psum = ctx.enter_context(tc.tile_pool(name="sc_psum", bufs=2, space="PSUM"))
        cx_psum = ctx.enter_context(tc.tile_pool(name="cx_psum", bufs=2, space="PSUM"))

        # ----- constants / preamble -----
        # consts go on the scalar (ACT) HWDGE ring so the big streaming loads
        # on sync/gpsimd aren't queued behind them; the first blocks' h_s
        # loads are issued by the main loop's prologue before most of these.
        identity = consts.tile([128, 128], F32, tag="identity")
        make_identity(nc, identity)
        ones_col = consts.tile([128, 1], F32, tag="ones_col")
        nc.vector.memset(ones_col, 1.0)
        ones_row = consts.tile([1, 128], F32, tag="ones_row")
        nc.vector.memset(ones_row, 1.0)
        eps_col = consts.tile([128, 1], F32, tag="eps_col")
        nc.vector.memset(eps_col, EPS)
        # trigger the ACT table load during the DMA-bound startup
        warm = consts.tile([128, 1], F32, tag="warm")
        nc.scalar.activation(warm[:, :], eps_col[:, :], Tanh)

        h_t_s = consts.tile([B, D], F32, tag="h_t_s")
        nc.scalar.dma_start(out=h_t_s[:, :], in_=h_t_e[:, :])
        W_a_s = consts.tile([D, H], F32, tag="W_a_s")
        nc.scalar.dma_start(out=W_a_s[:, :], in_=W_a_e[:, :])
        U_bf = consts.tile([D, H], BF16, tag="U_bf")
        nc.scalar.dma_start(out=U_bf[:, :], in_=U_a_e[:, :])
        V_bf = consts.tile([H, 1], BF16, tag="V_bf")
        nc.scalar.dma_start(out=V_bf[:, :], in_=V_a_e[:, :])
        Wc_top = consts.tile([D, H], F32, tag="Wc_top")
        nc.scalar.dma_start(out=Wc_top[:, :], in_=W_c_e[0:D, :])
        Wc_bot = consts.tile([D, H], F32, tag="Wc_bot")
        nc.scalar.dma_start(out=Wc_bot[:, :], in_=W_c_e[D : 2 * D, :])
        b_c_col = consts.tile([H, 1], F32, tag="b_c_col")
        nc.scalar.dma_start(out=b_c_col[:, :], in_=b_c_e[:])
        gamma_row = consts.tile([1, H], F32, tag="gamma_row")
        nc.scalar.dma_start(out=gamma_row[:, :], in_=gamma_e[:])
        beta_row = consts.tile([1, H], F32, tag="beta_row")
        nc.scalar.dma_start(out=beta_row[:, :], in_=beta_e[:])

        gamma_b = consts.tile([128, H], F32, tag="gamma_b")
        beta_b = consts.tile([128, H], F32, tag="beta_b")
        htT = consts.tile([D, B], F32, tag="htT")
        ht_projT = consts.tile([H, B], F32, tag="ht_projT")

        def emit_preamble_mms():
            # broadcast gamma/beta; htT = h_t^T; ht_projT = W_a^T htT.
            ps_g = sc_psum.tile([128, H], F32, tag="sc")
            nc.tensor.matmul(ps_g[:, :], lhsT=ones_row[:, :],
                             rhs=gamma_row[:, :], start=True, stop=True)
            nc.vector.tensor_copy(out=gamma_b[:, :], in_=ps_g[:, :])
            ps_b = sc_psum.tile([128, H], F32, tag="sc")
            nc.tensor.matmul(ps_b[:, :], lhsT=ones_row[:, :],
                             rhs=beta_row[:, :], start=True, stop=True)
            nc.vector.tensor_copy(out=beta_b[:, :], in_=ps_b[:, :])
            ps_t = sc_psum.tile([D, B], F32, tag="sc")
            nc.tensor.matmul(ps_t[:, :], lhsT=h_t_s[:, :], rhs=identity[:, :],
                             start=True, stop=True)
            nc.vector.tensor_copy(out=htT[:, :], in_=ps_t[:, :])
            ps_p = sc_psum.tile([H, B], F32, tag="sc")
            nc.tensor.matmul(ps_p[:, :], lhsT=W_a_s[:, :], rhs=htT[:, :],
                             start=True, stop=True)
            nc.vector.tensor_copy(out=ht_projT[:, :], in_=ps_p[:, :])

        # persistent accumulators
        ctxT_raw = consts.tile([D, B], F32, tag="ctxT_raw")
        sums_row = consts.tile([1, B], F32, tag="sums_row")

        # ----- pipelined main loop over blocks of BLK rows -----
        nat_tiles = {}    # block k -> nat tile [128, BLK, NCH, 128] bf16
        tanh_tiles = {}   # block k -> tanh tile [128, BLK, S] bf16
        grp_state = {}    # group g -> dict(ps_grp, expT, ps_ctx)

        def emit_dma(k):
            # both streams on the sync HWDGE ring (no SWDGE Q7 descriptor
            # generation, no SWDGE descriptor-ring SBUF contention); hsT is
            # emitted first so the critical projection input never queues
            # behind the larger nat transfer in the FIFO.
            hsT4 = hsT_pool.tile([128, BLK, S], F8, tag="hsT")
            nc.sync.dma_start(out=hsT4[:, :, :], in_=h_sT_e[:, k])
            nat4 = nat_pool.tile([128, BLK, NCH, D], BF16, tag="nat")
            nc.sync.dma_start(out=nat4[:, :, :, :], in_=h_s_e[:, k])
            nat_tiles[k] = nat4
            return hsT4

        def emit_proj_tanh(k, hsT4, filler):
            # `filler(t)` emits this slot's lagged score/ctx matmuls in
            # per-row portions between the psum-gated projection matmuls, so
            # the in-order TensorE stream always has ready work while the
            # next proj waits for its psum buffer to be evacuated.
            # The psum evacuation is the per-row tanh-with-bias on ACT:
            # a concurrent DVE read of a large psum tile throttles TensorE's
            # matmul stream 2-3x (measured), so DVE must never evacuate the
            # projection psum.
            tanh4 = tanh_pool.tile([128, BLK, S], F8, tag="tanh")
            for t in range(BLK):
                b = k * BLK + t
                ps_e = pe_psum.tile([H, S], F32, tag="pe")
                nc.tensor.matmul(ps_e[:, 0:512], lhsT=U_bf[:, :],
                                 rhs=hsT4[:, t, 0:512], start=True, stop=True)
                nc.tensor.matmul(ps_e[:, 512:1024], lhsT=U_bf[:, :],
                                 rhs=hsT4[:, t, 512:1024], start=True, stop=True)
                if k == 0 and t == 0:
                    emit_preamble_mms()
                if debug and b == 0:
                    nc.sync.dma_start(out=dbg_energy[:, :], in_=ps_e[:, :])
                nc.scalar.activation(tanh4[:, t, :], ps_e[:, :], Tanh,
                                     bias=ht_projT[:, b : b + 1], scale=1.0)
                filler(t)
            if debug and k == 0:
                nc.sync.dma_start(out=dbg_tanh[:, :], in_=tanh4[:, 0, :])
            tanh_tiles[k] = tanh4

        def emit_scores_row(k, t):
            # scores for row t of block k into group psum columns
            g = k // 2
            if k % 2 == 0 and t == 0:
                ps_grp = sc_psum.tile([128, GRP * NCH], F32, tag="sc")
                grp_state[g] = {"ps_grp": ps_grp}
            ps_grp = grp_state[g]["ps_grp"]
            tanh4 = tanh_tiles[k]
            b = k * BLK + t
            bl = b % GRP
            for c in range(NCH):
                nc.tensor.matmul(
                    ps_grp[:, bl * NCH + c : bl * NCH + c + 1],
                    lhsT=tanh4[:, t, c * 128 : (c + 1) * 128],
                    rhs=V_bf[:, :],
                    start=True, stop=True,
                )
            if t == BLK - 1:
                tanh_tiles.pop(k)

        def emit_group_head(g):
            # exp + per-row sums for group g (after all its scores)
            st = grp_state[g]
            ps_grp = st["ps_grp"]
            expT = exp_pool.tile([128, GRP * NCH], BF16, tag="expT")
            nc.scalar.activation(expT[:, :], ps_grp[:, :], Exp)
            if g == NG - 1:
                # last main-loop use of the exp/tanh table set: pull the
                # ln-set load off the epilogue critical path
                nc.scalar.activation(warm[:, :], eps_col[:, :], Ln)
            if debug and g == 0:
                nc.sync.dma_start(out=dbg_exp[:, :], in_=expT[:, :])
            st["expT"] = expT
            sumP = small.tile([128, GRP], F32, tag="sumP")
            nc.vector.tensor_reduce(
                sumP[:, :],
                expT.rearrange("p (b c) -> p b c", c=NCH),
                axis=AX_X, op=ADD,
            )
            ps_s = cx_psum.tile([128, GRP + 8], F32, tag="cx")
            nc.tensor.matmul(ps_s[0:1, GRP : 2 * GRP], lhsT=ones_col[:, :],
                             rhs=sumP[:, :], start=True, stop=True)
            sums_cp = nc.vector.tensor_copy(
                out=sums_row[:, g * GRP : (g + 1) * GRP],
                in_=ps_s[0:1, GRP : 2 * GRP])
            st["ps_ctx"] = ps_s
            st["sums_cp"] = sums_cp

        def emit_ctx_row(g, half, t):
            # context for row g*GRP + half*4 + t: nat chunks as weights,
            # exp column as rhs, accumulated over chunks in psum.
            st = grp_state[g]
            expT = st["expT"]
            ps_ctx = st["ps_ctx"]
            b = g * GRP + half * BLK + t
            bl = b % GRP
            k = b // BLK
            nat4 = nat_tiles[k]
            for c in range(NCH):
                mm = nc.tensor.matmul(
                    ps_ctx[:, bl : bl + 1],
                    lhsT=nat4[:, t, c, :],
                    rhs=expT[:, bl * NCH + c : bl * NCH + c + 1],
                    start=(c == 0), stop=(c == NCH - 1),
                )
                if "dep_done" not in st:
                    # first ctx matmul writes the same psum bank the DVE
                    # sums-copy reads; order them explicitly
                    bass._add_dep_helper(
                        mm.ins, st["sums_cp"].ins, sync=True,
                        reason="ctx writes wait for sums read")
                    st["dep_done"] = True
            if half == 1 and t == BLK - 1:
                nc.vector.tensor_copy(
                    out=ctxT_raw[:, g * GRP : (g + 1) * GRP],
                    in_=ps_ctx[:, 0:GRP])
                nat_tiles.pop(g * 2, None)
                nat_tiles.pop(g * 2 + 1, None)
                del grp_state[g]

        # prologue DMAs
        pend_hsT = {}
        for k in range(min(LAG + 1, NB)):
            pend_hsT[k] = emit_dma(k)

        # steady-state: one block of proj+tanh per slot, with the lagged
        # scores (slot k-LAG) and context rows (slot k-LAG-3) interleaved
        # row-by-row between the projection matmuls. Group heads run one
        # slot after their last scores, and context one slot after its
        # group head, so every cross-engine dependency is at least one
        # slot old and the in-order engine streams never stall on it.
        def make_filler(k):
            ks = k - LAG
            gh = k - LAG - 3

            def filler(t):
                if 0 <= ks < NB:
                    emit_scores_row(ks, t)
                if gh >= 0 and gh // 2 < NG:
                    emit_ctx_row(gh // 2, gh % 2, t)

            return filler

        for k in range(NB + LAG + 5):
            if k + LAG + 1 < NB:
                pend_hsT[k + LAG + 1] = emit_dma(k + LAG + 1)
            # group head first: its exp/sums dependencies finished last slot
            kgh = k - LAG - 1
            if kgh >= 0 and kgh % 2 == 1 and kgh // 2 < NG:
                emit_group_head(kgh // 2)
            filler = make_filler(k)
            if k < NB:
                emit_proj_tanh(k, pend_hsT.pop(k), filler)
            else:
                for t in range(BLK):
                    filler(t)

        # ----- epilogue -----
        nc.scalar.activation(warm[:, :], eps_col[:, :], Ln)
        inv_row = small.tile([1, B], F32, tag="inv_row")
        nc.vector.reciprocal(out=inv_row[:, :], in_=sums_row[:, :])
        ps_ib = sc_psum.tile([128, B], F32, tag="sc")
        nc.tensor.matmul(ps_ib[:, :], lhsT=ones_row[:, :], rhs=inv_row[:, :],
                         start=True, stop=True)
        ctxT = small.tile([D, B], F32, tag="ctxT")
        nc.vector.tensor_tensor(out=ctxT[:, :], in0=ctxT_raw[:, :],
                                in1=ps_ib[:, :], op=MULT)
        if debug:
            nc.sync.dma_start(out=dbg_sums[:, :], in_=sums_row[:, :])
            nc.sync.dma_start(out=dbg_ctx[:, :], in_=ctxT[:, :])
        ps_at = sc_psum.tile([H, B], F32, tag="sc")
        nc.tensor.matmul(ps_at[:, :], lhsT=Wc_top[:, :], rhs=ctxT[:, :],
                         start=True, stop=False)
        nc.tensor.matmul(ps_at[:, :], lhsT=Wc_bot[:, :], rhs=htT[:, :],
                         start=False, stop=True)
        attnT = small.tile([H, B], F32, tag="attnT")
        nc.scalar.activation(attnT[:, :], ps_at[:, :], Tanh,
                             bias=b_c_col[:, :], scale=1.0)
        ps_ab = sc_psum.tile([B, H], F32, tag="sc")
        nc.tensor.matmul(ps_ab[:, :], lhsT=attnT[:, :], rhs=identity[:, :],
                         start=True, stop=True)
        attn = small.tile([B, H], F32, tag="attn")
        nc.vector.tensor_copy(out=attn[:, :], in_=ps_ab[:, :])
        # LayerNorm over h (free dim), keras eps inside sqrt
        sum1 = small.tile([B, 1], F32, tag="sum1")
        nc.vector.tensor_reduce(sum1[:, :], attn[:, :], axis=AX_X, op=ADD)
        mean = small.tile([B, 1], F32, tag="mean")
        nc.vector.tensor_scalar_mul(mean[:, :], sum1[:, :], 1.0 / H)
        xc = small.tile([B, H], F32, tag="xc")
        nc.vector.tensor_scalar(out=xc[:, :], in0=attn[:, :],
                                scalar1=mean[:, :], scalar2=None, op0=SUB)
        sq = small.tile([B, H], F32, tag="sq")
        nc.vector.tensor_tensor(out=sq[:, :], in0=xc[:, :], in1=xc[:, :],
                                op=MULT)
        s2 = small.tile([B, 1], F32, tag="s2")
        nc.vector.tensor_reduce(s2[:, :], sq[:, :], axis=AX_X, op=ADD)
        var = small.tile([B, 1], F32, tag="var")
        nc.vector.tensor_scalar_mul(var[:, :], s2[:, :], 1.0 / H)
        # rstd = exp(-0.5 * ln(var + eps)) stays on the exp/ln table set
        lnv = small.tile([B, 1], F32, tag="lnv")
        nc.scalar.activation(lnv[:, :], var[:, :], Ln, bias=eps_col[:, :],
                             scale=1.0)
        istd = small.tile([B, 1], F32, tag="istd")
        nc.scalar.activation(istd[:, :], lnv[:, :], Exp, scale=-0.5)
        xn = small.tile([B, H], F32, tag="xn")
        nc.vector.tensor_scalar(out=xn[:, :], in0=xc[:, :],
                                scalar1=istd[:, :], scalar2=None, op0=MULT)
        y1 = small.tile([B, H], F32, tag="y1")
        nc.vector.tensor_tensor(out=y1[:, :], in0=xn[:, :], in1=gamma_b[:, :],
                                op=MULT)
        out_t = small.tile([B, H], F32, tag="out_t")
        nc.vector.tensor_tensor(out=out_t[:, :], in0=y1[:, :], in1=beta_b[:, :],
                                op=ADD)
        nc.sync.dma_start(out=out_e[:, :], in_=out_t[:, :])

    _normalize_waits(nc)
    return nc


def _normalize_waits(nc):
    """This walrus build rejects instructions carrying more sync waits than
    their ISA struct allows. Move excess waits onto single-wait nops
    immediately before the instruction on the same engine."""
    ZERO_WAIT = (mybir.InstDmaTransposeAnt, mybir.InstDrain)
    for blk in nc.main_func.blocks:
        insts = blk.instructions
        idx = 0
        while idx < len(insts):
            inst = insts[idx]
            si = inst.sync_info
            if si is not None:
                if isinstance(inst, ZERO_WAIT):
                    keep = 0
                elif isinstance(inst, mybir.InstEventSemaphore):
                    keep = 2
                else:
                    keep = 1
                waits = list(si.on_wait)
                if len(waits) > keep:
                    for w in waits[keep:]:
                        nop = mybir.InstNoOp(
                            name=nc.get_next_instruction_name(), ins=[], outs=[])
                        nop.engine = inst.engine
                        nop.sync_info = mybir.SyncInfo(on_wait=[w],
                                                       on_update=[])
                        nc.register_instruction(nop)
                        insts.insert(idx, nop)
                        idx += 1
                    si.on_wait = waits[:keep]
            idx += 1


_NC_CACHE = None


def _get_nc():
    global _NC_CACHE
    if _NC_CACHE is None:
        _NC_CACHE = _build()
    return _NC_CACHE


def _prep_host(h_s):
    """Host-side: build both h_s copies in the exact per-core SBUF tile
    layouts (one contiguous chunk per partition per block):
      nat [core, p, blk, t, r, d] bf16   with b = 4*blk + t, s = 8p + r
      hsT [core, d, blk, t, c, p] fp8    col j = c*128 + p  <->  s = 8p + c
    """
    import ml_dtypes
    Bf, Sf, Df = h_s.shape  # (1024, 1024, 128)
    x = h_s.reshape(NCORES, NB, BLK, 128, NCH, Df)
    nat = np.ascontiguousarray(x.transpose(0, 3, 1, 2, 4, 5)).astype(
        ml_dtypes.bfloat16)                     # [core, p, blk, t, r, d]
    hsT = np.ascontiguousarray(x.transpose(0, 5, 1, 2, 4, 3)).astype(
        ml_dtypes.float8_e4m3)                  # [core, d, blk, t, c, p]
    return nat, hsT


def _make_in_maps(h_t, h_s, W_a, U_a, V_a, W_c, b_c, gamma, beta):
    import ml_dtypes
    nat, hsT = _prep_host(np.asarray(h_s, dtype=np.float32))
    U_bf = np.asarray(U_a, dtype=ml_dtypes.bfloat16)
    V_bf = np.asarray(V_a, dtype=ml_dtypes.bfloat16)
    in_maps = []
    for i in range(NCORES):
        sl = slice(i * B, (i + 1) * B)
        in_maps.append({
            "h_t": np.ascontiguousarray(h_t[sl], dtype=np.float32),
            "h_s": nat[i].reshape(128, NB, BLK, NCH, D),
            "h_sT": hsT[i].reshape(128, NB, BLK, S),
            "W_a": np.ascontiguousarray(W_a, dtype=np.float32),
            "U_a": np.ascontiguousarray(U_bf),
            "V_a": np.ascontiguousarray(V_bf),
            "W_c": np.ascontiguousarray(W_c, dtype=np.float32),
            "b_c": np.ascontiguousarray(b_c, dtype=np.float32),
            "gamma": np.ascontiguousarray(gamma, dtype=np.float32),
            "beta": np.ascontiguousarray(beta, dtype=np.float32),
        })
    return in_maps


def run_spmd(trace=False, **inputs):
    """Runs the kernel; returns (full_output, BassKernelResults)."""
    nc = _get_nc()
    in_maps = _make_in_maps(**inputs)
    res = run_bass_kernel_spmd(nc, in_maps, core_ids=list(range(NCORES)),
                               trace=trace)
    out = np.concatenate([res.results[i]["out"] for i in range(NCORES)], axis=0)
    return out.astype(np.float32), res


def kernel(**inputs) -> np.ndarray:
    out, _ = run_spmd(trace=False, **inputs)
    return out
